# revision 50
# baseline (speedup 1.0000x reference)
"""Trainium2 Bass kernel for nn_BaselineTrustModel.

Math (see the reference): the per-timestep recurrence is affine and collapses
to a per-sample scalar formula.  With
    s    = sum_t perf[t, n]                (number of "fail" flags, 0..T)
    mask = any(obs[0, n, :] != 0)
    r1   = 1/sqrt(sigma0^2 + T*sigma_t^2)
    z0   = trust0/sqrt(sigma0^2)
    A    = (trust0 + T*wb + T*wtp) * r1
    B    = 2*wtp*r1
the output is
    pred[n] = clip(sigmoid(z0 + mask*( (A - z0) - B*s )), 0.01, 0.99)

Only obs[0] (N x D) and perf (T x N) are ever read -> ~66 MB of f32 input
traffic total, data-parallel over the sample axis N across 8 cores
(~8.3 MB per core, memory-bound; per-core HBM roofline ~358 GB/s -> ~23 us
of streaming; measured fixed preamble+tail of any NEFF here is ~13.5 us).

Device kernel per core (raw bacc, hand-scheduled; no TileContext).
Partition p owns samples [p*F, (p+1)*F), F = 490.  All tiles SBUF-resident;
every DMA dispatched with no buffer-reuse gating.  Engine split:

  Q7  : 16 perf t-layer cast-DMAs (SWDGE, f32 DRAM -> bf16 SBUF; perf
        values are 0/1 so the cast is exact).  SWDGE lanes add descriptor
        bandwidth alongside the two HWDGE queues.
  SP  : identity load + obs chunks 0,2,4 (HWDGE), the 2 stores.
  ACT : obs chunks 1,3 (its own HWDGE queue), table prewarm + 2 sigmoids.
  PE  : s = sum_t perf[t] as 16 PSUM-accumulated identity matmuls
        (I.T @ l_t accumulated; bf16 x bf16 -> f32 PSUM, exact).
  DVE : 5 segmented abs-max obs reduces, dd = s*(-B)+(A-z0) straight from
        PSUM, x = (ma>0)*dd, clip halves (pipelined with ACT sigmoids).
"""

import math
import sys
from contextlib import ExitStack

import numpy as np

for _p in ("/opt/trn_rl_repo", "/root/.axon_site/_ro/trn_rl_repo"):
    if _p not in sys.path:
        sys.path.append(_p)

T = 16
D = 16
N = 500000
NCORES = 8

F = 490            # samples per partition per core
K = 5              # obs chunks (F % K == 0)
MH = F // 2        # epilogue half width
PER = 128 * F      # 62720 samples per core
NPAD = NCORES * PER


def build_program(neg_b, c_const, z0):
    """Raw-bacc single-core program (SPMD across cores)."""
    from concourse import bacc, mybir

    f32 = mybir.dt.float32
    bf16 = mybir.dt.bfloat16
    fc = F // K                      # 98 samples per obs chunk per partition
    nc = bacc.Bacc("TRN2", target_bir_lowering=False, debug=False)
    obs_d = nc.dram_tensor("obs0", [128, K, fc * D], f32, kind="ExternalInput").ap()
    perf_d = nc.dram_tensor("perfc", [T, 128, F], f32, kind="ExternalInput").ap()
    id_d = nc.dram_tensor("ident", [128, 128], bf16, kind="ExternalInput").ap()
    out_d = nc.dram_tensor("out", [128, F], f32, kind="ExternalOutput").ap()

    with ExitStack() as ctx:
        pb = [
            ctx.enter_context(nc.sbuf_tensor(f"pb{i}", [128, F], bf16))
            for i in range(T)
        ]
        sbf = lambda name, shape: ctx.enter_context(nc.sbuf_tensor(name, shape, f32))
        ob = [sbf(f"ob{k}", [128, fc * D]) for k in range(K)]
        ident = ctx.enter_context(nc.sbuf_tensor("idnt", [128, 128], bf16))
        ma = sbf("ma", [128, F])
        dd = sbf("dd", [128, F])
        xx = sbf("xx", [128, F])
        pp = sbf("pp", [128, F])
        oo = sbf("oo", [128, F])
        z0t = sbf("z0t", [128, 1])
        scr = sbf("scr", [128, 1])
        ps = ctx.enter_context(nc.psum_tensor("ps", [128, F], f32))

        pdma = [ctx.enter_context(nc.semaphore(f"pd{i}")) for i in range(T)]
        obdma = [ctx.enter_context(nc.semaphore(f"od{k}")) for k in range(K)]
        iddma = ctx.enter_context(nc.semaphore("iddma"))
        odma = ctx.enter_context(nc.semaphore("odma"))
        dve = ctx.enter_context(nc.semaphore("dve"))
        pe = ctx.enter_context(nc.semaphore("pe"))
        act = ctx.enter_context(nc.semaphore("act"))
        all_sems = pdma + obdma + [iddma, odma, dve, pe, act]
        nums = sorted(s.num for s in all_sems)
        assert nums == list(range(nums[0], nums[0] + len(nums))), nums
        sem_range = range(nums[0], nums[-1] + 1)

        block_cm = nc.Block()
        block = block_cm.__enter__()

        marks = {}  # landmark name -> dve counter value

        @block.gpsimd
        def _(gpsimd):
            for i in range(T):
                gpsimd.dma_start(pb[i][:], perf_d[i]).then_inc(pdma[i], 16)

        @block.tensor
        def _(tensor):
            tensor.wait_ge(iddma, 16)
            for i in range(T):
                tensor.wait_ge(pdma[i], 16)
                nc.tensor.matmul(
                    ps[:], ident[:], pb[i][:],
                    start=(i == 0), stop=(i == T - 1),
                ).then_inc(pe, 1)

        @block.vector
        def _(vector):
            cnt = [0]

            def emit(instr, mark=None):
                instr.then_inc(dve, 1)
                cnt[0] += 1
                if mark:
                    marks[mark] = cnt[0]
                return cnt[0]

            emit(nc.vector.memset(z0t[:], z0), mark="z0")
            for k in range(K):
                vector.wait_ge(obdma[k], 16)
                emit(nc.vector.tensor_reduce(
                    ma[:, k * fc:(k + 1) * fc],
                    ob[k][:].rearrange("p (f d) -> p f d", d=D),
                    axis=mybir.AxisListType.X,
                    op=mybir.AluOpType.max,
                    apply_absolute_value=True,
                ))
            vector.wait_ge(pe, T)
            emit(nc.vector.tensor_scalar(
                dd[:], ps[:], neg_b, c_const,
                op0=mybir.AluOpType.mult, op1=mybir.AluOpType.add,
            ))
            vector.wait_ge(dve, cnt[0])
            emit(nc.vector.scalar_tensor_tensor(
                xx[:], ma[:], 0.0, dd[:],
                op0=mybir.AluOpType.is_gt, op1=mybir.AluOpType.mult,
            ), mark="x")
            for h in range(2):
                vector.wait_ge(act, h + 2)
                emit(nc.vector.tensor_scalar(
                    oo[:, h * MH:(h + 1) * MH], pp[:, h * MH:(h + 1) * MH],
                    0.01, 0.99,
                    op0=mybir.AluOpType.max, op1=mybir.AluOpType.min,
                ), mark=f"clip{h}")

        @block.sync
        def _(sync):
            sync.dma_start(ident[:], id_d).then_inc(iddma, 16)
            for k in (0, 2, 4):
                sync.dma_start(ob[k][:], obs_d[:, k]).then_inc(obdma[k], 16)
            sync.wait_ge(dve, marks["clip0"])
            sync.dma_start(out_d[:, 0:MH], oo[:, 0:MH]).then_inc(odma, 16)
            sync.wait_ge(dve, marks["clip1"])
            sync.dma_start(out_d[:, MH:F], oo[:, MH:F]).then_inc(odma, 16)
            sync.wait_ge(odma, 32)

        @block.scalar
        def _(scalar):
            for k in (1, 3):
                scalar.dma_start(ob[k][:], obs_d[:, k]).then_inc(obdma[k], 16)
            # prewarm the sigmoid table set while the stream runs
            scalar.wait_ge(dve, marks["z0"])
            nc.scalar.activation(
                scr[:], z0t[:], mybir.ActivationFunctionType.Sigmoid,
            ).then_inc(act, 1)
            scalar.wait_ge(dve, marks["x"])
            for h in range(2):
                nc.scalar.activation(
                    pp[:, h * MH:(h + 1) * MH], xx[:, h * MH:(h + 1) * MH],
                    mybir.ActivationFunctionType.Sigmoid,
                    bias=z0t[:], scale=1.0,
                ).then_inc(act, 1)

        block_cm.__exit__(None, None, None)
        # Re-executable NEFF tail (the NTFF profiler replays it).
        nc.all_engine_barrier()
        nc.gpsimd.dma_reset(sem_range)
        nc.gpsimd.sem_clear(sem_range)

    nc.compile()
    return nc


def _scalar_constants(inputs):
    t0 = float(np.asarray(inputs["trust0"]).reshape(()))
    s0 = float(np.asarray(inputs["sigma0"]).reshape(()))
    wb = float(np.asarray(inputs["wb"]).reshape(()))
    wtp = float(np.asarray(inputs["wtp"]).reshape(()))
    st = float(np.asarray(inputs["sigma_t"]).reshape(()))
    r1 = 1.0 / math.sqrt(s0 * s0 + T * st * st)
    z0 = t0 / math.sqrt(s0 * s0)
    a_const = (t0 + T * wb + T * wtp) * r1
    neg_b = -2.0 * wtp * r1
    c_const = a_const - z0
    return neg_b, c_const, z0


def run(inputs, trace=False, **kw):
    """Shard, run on 8 cores, gather. Returns (output [N,1] f32, exec_time_ns)."""
    import ml_dtypes
    from concourse.bass_utils import run_bass_kernel_spmd

    obs = np.asarray(inputs["inptasksobs"])
    perf = np.asarray(inputs["inptasksperf"])
    assert obs.shape == (T, N, D) and perf.shape == (T, N, 1)

    neg_b, c_const, z0 = _scalar_constants(inputs)
    nc = build_program(neg_b, c_const, z0)

    obs_p = np.zeros((NPAD, D), np.float32)
    obs_p[:N] = obs[0]
    perf_p = np.zeros((T, NPAD), np.float32)
    perf_p[:, :N] = perf[:, :, 0]
    ident = np.eye(128, dtype=ml_dtypes.bfloat16)

    in_maps = []
    for c in range(NCORES):
        oc = obs_p[c * PER:(c + 1) * PER].reshape(128, K, (F // K) * D)
        pc = np.ascontiguousarray(
            perf_p[:, c * PER:(c + 1) * PER]
        ).reshape(T, 128, F)
        in_maps.append({"obs0": oc, "perfc": pc, "ident": ident})

    res = run_bass_kernel_spmd(
        nc, in_maps, core_ids=list(range(NCORES)), trace=trace, **kw
    )
    full = np.concatenate(
        [res.results[c]["out"].reshape(-1) for c in range(NCORES)]
    )
    return full[:N].reshape(N, 1).astype(np.float32, copy=False), res.exec_time_ns


def kernel(**inputs):
    out, _ = run(inputs, trace=False)
    return out


# revision 52
# speedup vs baseline: 1.0121x; 1.0121x over previous
"""Trainium2 Bass kernel for nn_BaselineTrustModel.

Math (see the reference): the per-timestep recurrence is affine and collapses
to a per-sample scalar formula.  With
    s    = sum_t perf[t, n]                (number of "fail" flags, 0..T)
    mask = any(obs[0, n, :] != 0)
    r1   = 1/sqrt(sigma0^2 + T*sigma_t^2)
    z0   = trust0/sqrt(sigma0^2)
    A    = (trust0 + T*wb + T*wtp) * r1
    B    = 2*wtp*r1
the output is
    pred[n] = clip(sigmoid(z0 + mask*( (A - z0) - B*s )), 0.01, 0.99)

Only obs[0] (N x D) and perf (T x N) are ever read -> ~66 MB of f32 input
traffic total, data-parallel over the sample axis N across 8 cores
(~8.3 MB per core, memory-bound; per-core HBM roofline ~358 GB/s -> ~23 us
of streaming; measured fixed preamble+tail of any NEFF here is ~13.5 us).

Device kernel per core (raw bacc, hand-scheduled; no TileContext).
Partition p owns samples [p*F, (p+1)*F), F = 490.  All tiles SBUF-resident;
every DMA dispatched with no buffer-reuse gating.  Engine split:

  Q7  : 16 perf t-layer cast-DMAs (SWDGE, f32 DRAM -> bf16 SBUF; perf
        values are 0/1 so the cast is exact).  SWDGE lanes add descriptor
        bandwidth alongside the two HWDGE queues.
  SP  : identity load + obs chunks 0,2,4 (HWDGE), the 2 stores.
  ACT : obs chunks 1,3 (its own HWDGE queue), table prewarm + 2 sigmoids.
  PE  : s = sum_t perf[t] as 16 PSUM-accumulated identity matmuls
        (I.T @ l_t accumulated; bf16 x bf16 -> f32 PSUM, exact).
  DVE : 5 segmented abs-max obs reduces, dd = s*(-B)+(A-z0) straight from
        PSUM, x = (ma>0)*dd, clip halves (pipelined with ACT sigmoids).
"""

import math
import sys
from contextlib import ExitStack

import numpy as np

for _p in ("/opt/trn_rl_repo", "/root/.axon_site/_ro/trn_rl_repo"):
    if _p not in sys.path:
        sys.path.append(_p)

T = 16
D = 16
N = 500000
NCORES = 8

F = 490            # samples per partition per core
K = 5              # obs chunks (F % K == 0)
MH = F // 2        # epilogue half width
PER = 128 * F      # 62720 samples per core
NPAD = NCORES * PER


def build_program(neg_b, c_const, z0):
    """Raw-bacc single-core program (SPMD across cores)."""
    from concourse import bacc, mybir

    f32 = mybir.dt.float32
    bf16 = mybir.dt.bfloat16
    fc = F // K                      # 98 samples per obs chunk per partition
    nc = bacc.Bacc("TRN2", target_bir_lowering=False, debug=False)
    obs_d = nc.dram_tensor("obs0", [128, K, fc * D], f32, kind="ExternalInput").ap()
    perf_d = nc.dram_tensor("perfc", [T, 128, F], f32, kind="ExternalInput").ap()
    id_d = nc.dram_tensor("ident", [128, 128], bf16, kind="ExternalInput").ap()
    out_d = nc.dram_tensor("out", [128, F], f32, kind="ExternalOutput").ap()

    with ExitStack() as ctx:
        pb = [
            ctx.enter_context(nc.sbuf_tensor(f"pb{i}", [128, F], bf16))
            for i in range(T)
        ]
        sbf = lambda name, shape: ctx.enter_context(nc.sbuf_tensor(name, shape, f32))
        ob = [sbf(f"ob{k}", [128, fc * D]) for k in range(K)]
        ident = ctx.enter_context(nc.sbuf_tensor("idnt", [128, 128], bf16))
        ma = sbf("ma", [128, F])
        dd = sbf("dd", [128, F])
        xx = sbf("xx", [128, F])
        pp = sbf("pp", [128, F])
        oo = sbf("oo", [128, F])
        z0t = sbf("z0t", [128, 1])
        scr = sbf("scr", [128, 1])
        ps = ctx.enter_context(nc.psum_tensor("ps", [128, F], f32))

        pdma = [ctx.enter_context(nc.semaphore(f"pd{i}")) for i in range(T)]
        obdma = [ctx.enter_context(nc.semaphore(f"od{k}")) for k in range(K)]
        iddma = ctx.enter_context(nc.semaphore("iddma"))
        odma = ctx.enter_context(nc.semaphore("odma"))
        dve = ctx.enter_context(nc.semaphore("dve"))
        pe = ctx.enter_context(nc.semaphore("pe"))
        act = ctx.enter_context(nc.semaphore("act"))
        all_sems = pdma + obdma + [iddma, odma, dve, pe, act]
        nums = sorted(s.num for s in all_sems)
        assert nums == list(range(nums[0], nums[0] + len(nums))), nums
        sem_range = range(nums[0], nums[-1] + 1)

        block_cm = nc.Block()
        block = block_cm.__enter__()

        marks = {}  # landmark name -> dve counter value

        @block.gpsimd
        def _(gpsimd):
            for i in range(T):
                gpsimd.dma_start(pb[i][:], perf_d[i]).then_inc(pdma[i], 16)

        @block.tensor
        def _(tensor):
            tensor.wait_ge(iddma, 16)
            for i in range(T):
                tensor.wait_ge(pdma[i], 16)
                nc.tensor.matmul(
                    ps[:], ident[:], pb[i][:],
                    start=(i == 0), stop=(i == T - 1),
                ).then_inc(pe, 1)

        @block.vector
        def _(vector):
            cnt = [0]

            def emit(instr, mark=None):
                instr.then_inc(dve, 1)
                cnt[0] += 1
                if mark:
                    marks[mark] = cnt[0]
                return cnt[0]

            emit(nc.vector.memset(z0t[:], z0), mark="z0")
            for k in range(K):
                vector.wait_ge(obdma[k], 16)
                emit(nc.vector.tensor_reduce(
                    ma[:, k * fc:(k + 1) * fc],
                    ob[k][:].rearrange("p (f d) -> p f d", d=D),
                    axis=mybir.AxisListType.X,
                    op=mybir.AluOpType.max,
                    apply_absolute_value=True,
                ))
            vector.wait_ge(pe, T)
            for h in range(2):
                sl = slice(h * MH, (h + 1) * MH)
                emit(nc.vector.tensor_scalar(
                    dd[:, sl], ps[:, sl], neg_b, c_const,
                    op0=mybir.AluOpType.mult, op1=mybir.AluOpType.add,
                ))
                vector.wait_ge(dve, cnt[0])
                emit(nc.vector.scalar_tensor_tensor(
                    xx[:, sl], ma[:, sl], 0.0, dd[:, sl],
                    op0=mybir.AluOpType.is_gt, op1=mybir.AluOpType.mult,
                ), mark=f"x{h}")
            for h in range(2):
                vector.wait_ge(act, h + 2)
                emit(nc.vector.tensor_scalar(
                    oo[:, h * MH:(h + 1) * MH], pp[:, h * MH:(h + 1) * MH],
                    0.01, 0.99,
                    op0=mybir.AluOpType.max, op1=mybir.AluOpType.min,
                ), mark=f"clip{h}")

        @block.sync
        def _(sync):
            sync.dma_start(ident[:], id_d).then_inc(iddma, 16)
            for k in (0, 2, 4):
                sync.dma_start(ob[k][:], obs_d[:, k]).then_inc(obdma[k], 16)
            sync.wait_ge(dve, marks["clip0"])
            sync.dma_start(out_d[:, 0:MH], oo[:, 0:MH]).then_inc(odma, 16)
            sync.wait_ge(dve, marks["clip1"])
            sync.dma_start(out_d[:, MH:F], oo[:, MH:F]).then_inc(odma, 16)
            sync.wait_ge(odma, 32)

        @block.scalar
        def _(scalar):
            for k in (1, 3):
                scalar.dma_start(ob[k][:], obs_d[:, k]).then_inc(obdma[k], 16)
            # prewarm the sigmoid table set while the stream runs
            scalar.wait_ge(dve, marks["z0"])
            nc.scalar.activation(
                scr[:], z0t[:], mybir.ActivationFunctionType.Sigmoid,
            ).then_inc(act, 1)
            for h in range(2):
                scalar.wait_ge(dve, marks[f"x{h}"])
                nc.scalar.activation(
                    pp[:, h * MH:(h + 1) * MH], xx[:, h * MH:(h + 1) * MH],
                    mybir.ActivationFunctionType.Sigmoid,
                    bias=z0t[:], scale=1.0,
                ).then_inc(act, 1)

        block_cm.__exit__(None, None, None)
        # Re-executable NEFF tail (the NTFF profiler replays it).
        nc.all_engine_barrier()
        nc.gpsimd.dma_reset(sem_range)
        nc.gpsimd.sem_clear(sem_range)

    nc.compile()
    return nc


def _scalar_constants(inputs):
    t0 = float(np.asarray(inputs["trust0"]).reshape(()))
    s0 = float(np.asarray(inputs["sigma0"]).reshape(()))
    wb = float(np.asarray(inputs["wb"]).reshape(()))
    wtp = float(np.asarray(inputs["wtp"]).reshape(()))
    st = float(np.asarray(inputs["sigma_t"]).reshape(()))
    r1 = 1.0 / math.sqrt(s0 * s0 + T * st * st)
    z0 = t0 / math.sqrt(s0 * s0)
    a_const = (t0 + T * wb + T * wtp) * r1
    neg_b = -2.0 * wtp * r1
    c_const = a_const - z0
    return neg_b, c_const, z0


def run(inputs, trace=False, **kw):
    """Shard, run on 8 cores, gather. Returns (output [N,1] f32, exec_time_ns)."""
    import ml_dtypes
    from concourse.bass_utils import run_bass_kernel_spmd

    obs = np.asarray(inputs["inptasksobs"])
    perf = np.asarray(inputs["inptasksperf"])
    assert obs.shape == (T, N, D) and perf.shape == (T, N, 1)

    neg_b, c_const, z0 = _scalar_constants(inputs)
    nc = build_program(neg_b, c_const, z0)

    obs_p = np.zeros((NPAD, D), np.float32)
    obs_p[:N] = obs[0]
    perf_p = np.zeros((T, NPAD), np.float32)
    perf_p[:, :N] = perf[:, :, 0]
    ident = np.eye(128, dtype=ml_dtypes.bfloat16)

    in_maps = []
    for c in range(NCORES):
        oc = obs_p[c * PER:(c + 1) * PER].reshape(128, K, (F // K) * D)
        pc = np.ascontiguousarray(
            perf_p[:, c * PER:(c + 1) * PER]
        ).reshape(T, 128, F)
        in_maps.append({"obs0": oc, "perfc": pc, "ident": ident})

    res = run_bass_kernel_spmd(
        nc, in_maps, core_ids=list(range(NCORES)), trace=trace, **kw
    )
    full = np.concatenate(
        [res.results[c]["out"].reshape(-1) for c in range(NCORES)]
    )
    return full[:N].reshape(N, 1).astype(np.float32, copy=False), res.exec_time_ns


def kernel(**inputs):
    out, _ = run(inputs, trace=False)
    return out


# revision 55
# speedup vs baseline: 1.1221x; 1.1087x over previous
"""Trainium2 Bass kernel for nn_BaselineTrustModel.

Math (see the reference): the per-timestep recurrence is affine and collapses
to a per-sample scalar formula.  With
    s    = sum_t perf[t, n]                (number of "fail" flags, 0..T)
    mask = any(obs[0, n, :] != 0)
    r1   = 1/sqrt(sigma0^2 + T*sigma_t^2)
    z0   = trust0/sqrt(sigma0^2)
    A    = (trust0 + T*wb + T*wtp) * r1
    B    = 2*wtp*r1
the output is
    pred[n] = clip(sigmoid(z0 + mask*( (A - z0) - B*s )), 0.01, 0.99)

Only obs[0] (N x D) and perf (T x N) are ever read -> ~66 MB of f32 input
traffic total, data-parallel over the sample axis N across 8 cores
(~8.3 MB per core, memory-bound; per-core HBM roofline ~358 GB/s -> ~23 us
of streaming; measured fixed preamble+tail of any NEFF here is ~13.5 us).

Device kernel per core (raw bacc, hand-scheduled; no TileContext).
Partition p owns samples [p*F, (p+1)*F), F = 490.  All tiles SBUF-resident;
every DMA dispatched with no buffer-reuse gating.  Engine split:

  Q7  : 16 perf t-layer cast-DMAs (SWDGE, f32 DRAM -> bf16 SBUF; perf
        values are 0/1 so the cast is exact).  SWDGE lanes add descriptor
        bandwidth alongside the two HWDGE queues.
  SP  : identity load + obs chunks 0,2,4 (HWDGE), the 2 stores.
  ACT : obs chunks 1,3 (its own HWDGE queue), table prewarm + 2 sigmoids.
  PE  : s = sum_t perf[t] as 16 PSUM-accumulated identity matmuls
        (I.T @ l_t accumulated; bf16 x bf16 -> f32 PSUM, exact).
  DVE : 5 segmented abs-max obs reduces, dd = s*(-B)+(A-z0) straight from
        PSUM, x = (ma>0)*dd, clip halves (pipelined with ACT sigmoids).
"""

import math
import sys
from contextlib import ExitStack

import numpy as np

for _p in ("/opt/trn_rl_repo", "/root/.axon_site/_ro/trn_rl_repo"):
    if _p not in sys.path:
        sys.path.append(_p)

T = 16
D = 16
N = 500000
NCORES = 8

F = 490            # samples per partition per core
K = 5              # obs chunks (F % K == 0)
MH = F // 2        # epilogue half width
PER = 128 * F      # 62720 samples per core
NPAD = NCORES * PER


def build_program(neg_b, c_const, z0):
    """Raw-bacc single-core program (SPMD across cores)."""
    from concourse import bacc, mybir

    f32 = mybir.dt.float32
    bf16 = mybir.dt.bfloat16
    fc = F // K                      # 98 samples per obs chunk per partition
    nc = bacc.Bacc("TRN2", target_bir_lowering=False, debug=False)
    obs_d = nc.dram_tensor("obs0", [128, K, fc * D], f32, kind="ExternalInput").ap()
    perf_d = nc.dram_tensor("perfc", [T, 128, F], f32, kind="ExternalInput").ap()
    id_d = nc.dram_tensor("ident", [128, 128], bf16, kind="ExternalInput").ap()
    out_d = nc.dram_tensor("out", [128, F], f32, kind="ExternalOutput").ap()

    with ExitStack() as ctx:
        pb = [
            ctx.enter_context(nc.sbuf_tensor(f"pb{i}", [128, F], bf16))
            for i in range(T)
        ]
        sbf = lambda name, shape: ctx.enter_context(nc.sbuf_tensor(name, shape, f32))
        ob = [sbf(f"ob{k}", [128, fc * D]) for k in range(K)]
        ident = ctx.enter_context(nc.sbuf_tensor("idnt", [128, 128], bf16))
        ma = sbf("ma", [128, F])
        dd = sbf("dd", [128, F])
        xx = sbf("xx", [128, F])
        pp = sbf("pp", [128, F])
        oo = sbf("oo", [128, F])
        z0t = sbf("z0t", [128, 1])
        scr = sbf("scr", [128, 1])
        ps = ctx.enter_context(nc.psum_tensor("ps", [128, F], f32))

        pdma = [ctx.enter_context(nc.semaphore(f"pd{i}")) for i in range(T)]
        obdma = [ctx.enter_context(nc.semaphore(f"od{k}")) for k in range(K)]
        iddma = ctx.enter_context(nc.semaphore("iddma"))
        odma = ctx.enter_context(nc.semaphore("odma"))
        dve = ctx.enter_context(nc.semaphore("dve"))
        pe = ctx.enter_context(nc.semaphore("pe"))
        act = ctx.enter_context(nc.semaphore("act"))
        all_sems = pdma + obdma + [iddma, odma, dve, pe, act]
        nums = sorted(s.num for s in all_sems)
        assert nums == list(range(nums[0], nums[0] + len(nums))), nums
        sem_range = range(nums[0], nums[-1] + 1)

        block_cm = nc.Block()
        block = block_cm.__enter__()

        marks = {}  # landmark name -> dve counter value

        @block.gpsimd
        def _(gpsimd):
            for i in range(T):
                gpsimd.dma_start(pb[i][:], perf_d[i]).then_inc(pdma[i], 16)

        @block.tensor
        def _(tensor):
            tensor.wait_ge(iddma, 16)
            for i in range(T):
                tensor.wait_ge(pdma[i], 16)
                nc.tensor.matmul(
                    ps[:], ident[:], pb[i][:],
                    start=(i == 0), stop=(i == T - 1),
                ).then_inc(pe, 1)

        @block.vector
        def _(vector):
            cnt = [0]

            def emit(instr, mark=None):
                instr.then_inc(dve, 1)
                cnt[0] += 1
                if mark:
                    marks[mark] = cnt[0]
                return cnt[0]

            emit(nc.vector.memset(z0t[:], z0), mark="z0")
            for k in range(K):
                vector.wait_ge(obdma[k], 16)
                emit(nc.vector.tensor_reduce(
                    ma[:, k * fc:(k + 1) * fc],
                    ob[k][:].rearrange("p (f d) -> p f d", d=D),
                    axis=mybir.AxisListType.X,
                    op=mybir.AluOpType.max,
                    apply_absolute_value=True,
                ))
            # clip(sigmoid(z), .01, .99) == sigmoid(clamp(z, logit(.01),
            # logit(.99))) to ~1e-7; clamping in z-space removes the
            # post-sigmoid DVE clip (and its ACT->DVE->SP tail hop).
            xlo = math.log(0.01 / 0.99) - z0
            xhi = math.log(0.99 / 0.01) - z0
            vector.wait_ge(pe, T)
            for h in range(2):
                sl = slice(h * MH, (h + 1) * MH)
                emit(nc.vector.tensor_scalar(
                    dd[:, sl], ps[:, sl], neg_b, c_const,
                    op0=mybir.AluOpType.mult, op1=mybir.AluOpType.add,
                ))
                vector.wait_ge(dve, cnt[0])
                emit(nc.vector.scalar_tensor_tensor(
                    xx[:, sl], ma[:, sl], 0.0, dd[:, sl],
                    op0=mybir.AluOpType.is_gt, op1=mybir.AluOpType.mult,
                ))
                vector.wait_ge(dve, cnt[0])
                emit(nc.vector.tensor_scalar(
                    oo[:, sl], xx[:, sl], xlo, xhi,
                    op0=mybir.AluOpType.max, op1=mybir.AluOpType.min,
                ), mark=f"x{h}")

        @block.sync
        def _(sync):
            sync.dma_start(ident[:], id_d).then_inc(iddma, 16)
            for k in (0, 2, 4):
                sync.dma_start(ob[k][:], obs_d[:, k]).then_inc(obdma[k], 16)
            sync.wait_ge(act, 2)
            sync.dma_start(out_d[:, 0:MH], pp[:, 0:MH]).then_inc(odma, 16)
            sync.wait_ge(act, 3)
            sync.dma_start(out_d[:, MH:F], pp[:, MH:F]).then_inc(odma, 16)
            sync.wait_ge(odma, 32)

        @block.scalar
        def _(scalar):
            for k in (1, 3):
                scalar.dma_start(ob[k][:], obs_d[:, k]).then_inc(obdma[k], 16)
            # prewarm the sigmoid table set while the stream runs
            scalar.wait_ge(dve, marks["z0"])
            nc.scalar.activation(
                scr[:], z0t[:], mybir.ActivationFunctionType.Sigmoid,
            ).then_inc(act, 1)
            for h in range(2):
                scalar.wait_ge(dve, marks[f"x{h}"])
                nc.scalar.activation(
                    pp[:, h * MH:(h + 1) * MH], oo[:, h * MH:(h + 1) * MH],
                    mybir.ActivationFunctionType.Sigmoid,
                    bias=z0t[:], scale=1.0,
                ).then_inc(act, 1)

        block_cm.__exit__(None, None, None)
        # Re-executable NEFF tail (the NTFF profiler replays it).
        nc.all_engine_barrier()
        nc.gpsimd.dma_reset(sem_range)
        nc.gpsimd.sem_clear(sem_range)

    nc.compile()
    return nc


def _scalar_constants(inputs):
    t0 = float(np.asarray(inputs["trust0"]).reshape(()))
    s0 = float(np.asarray(inputs["sigma0"]).reshape(()))
    wb = float(np.asarray(inputs["wb"]).reshape(()))
    wtp = float(np.asarray(inputs["wtp"]).reshape(()))
    st = float(np.asarray(inputs["sigma_t"]).reshape(()))
    r1 = 1.0 / math.sqrt(s0 * s0 + T * st * st)
    z0 = t0 / math.sqrt(s0 * s0)
    a_const = (t0 + T * wb + T * wtp) * r1
    neg_b = -2.0 * wtp * r1
    c_const = a_const - z0
    return neg_b, c_const, z0


def run(inputs, trace=False, **kw):
    """Shard, run on 8 cores, gather. Returns (output [N,1] f32, exec_time_ns)."""
    import ml_dtypes
    from concourse.bass_utils import run_bass_kernel_spmd

    obs = np.asarray(inputs["inptasksobs"])
    perf = np.asarray(inputs["inptasksperf"])
    assert obs.shape == (T, N, D) and perf.shape == (T, N, 1)

    neg_b, c_const, z0 = _scalar_constants(inputs)
    nc = build_program(neg_b, c_const, z0)

    obs_p = np.zeros((NPAD, D), np.float32)
    obs_p[:N] = obs[0]
    perf_p = np.zeros((T, NPAD), np.float32)
    perf_p[:, :N] = perf[:, :, 0]
    ident = np.eye(128, dtype=ml_dtypes.bfloat16)

    in_maps = []
    for c in range(NCORES):
        oc = obs_p[c * PER:(c + 1) * PER].reshape(128, K, (F // K) * D)
        pc = np.ascontiguousarray(
            perf_p[:, c * PER:(c + 1) * PER]
        ).reshape(T, 128, F)
        in_maps.append({"obs0": oc, "perfc": pc, "ident": ident})

    res = run_bass_kernel_spmd(
        nc, in_maps, core_ids=list(range(NCORES)), trace=trace, **kw
    )
    full = np.concatenate(
        [res.results[c]["out"].reshape(-1) for c in range(NCORES)]
    )
    return full[:N].reshape(N, 1).astype(np.float32, copy=False), res.exec_time_ns


def kernel(**inputs):
    out, _ = run(inputs, trace=False)
    return out
